# revision 7
# baseline (speedup 1.0000x reference)
"""Trainium2 Bass kernel for nn_BasicBlock (gnn_message_passing).

kernel(**inputs) takes the FULL unsharded inputs
  x [4,128,65536] f32, coords [4,3,65536] f32, indices/reindices [4,65536]
  i32, w1/w2 [128,128,9] f32, gamma/beta [128] f32
and returns the FULL output [4,128,65536] f32.

Sharding: data-parallel over batch x curve-half across 8 NeuronCores
(core k: batch k//2, half k%2, +-halo overlap). All permutation
gathers/scatters run on device via indirect DMA; BN batch stats are
all-reduced on device with a collective over all 8 cores.

Per-core math (curve order; gather/scatter commute with BN/ReLU):
  y1 = conv_g(x, w1); h = relu(a1*y1 + b1); y2 = conv_g(h, w2)
  out = relu(a2*y2 + b2' + x), scattered back through indices.
  conv_g(z)[:, n] = sum_t w[:, :, t] @ (z[:, n+t-4] * g[t, n]),
  g[t, n] = exp(-|c[n+t-4] - c[n]|^2); g[4, :] == 1 and
  g[8-t, n] = g[t, n+4-t], so only taps 0..3 need replicated scales.
OOB sentinel rows: x-row = 0 and coords-row = 1e3, so any tap touching an
out-of-range source gets g ~ exp(-1e6) = 0 (emulates the reference's
zero-padding exactly).
"""

import sys
import numpy as np
from contextlib import ExitStack

sys.path.insert(0, "/opt/trn_rl_repo")

import ml_dtypes
import concourse.bass as bass
import concourse.tile as tile
from concourse import bacc, mybir
from concourse.bass import IndirectOffsetOnAxis
from concourse.bass_utils import run_bass_kernel_spmd

F32 = mybir.dt.float32
BF16 = mybir.dt.bfloat16
I32 = mybir.dt.int32
AF = mybir.ActivationFunctionType
ALU = mybir.AluOpType
AX = mybir.AxisListType

C = 128
K = 9
PAD = 4
HALO = 8

N_FULL = 65536
B_FULL = 4
N_CORES = 8


def ceil_div(a, b):
    return (a + b - 1) // b


class Cfg:
    def __init__(self, N, n_cores, L=1024, GL=1024, KB=8, dbg=False):
        self.dbg = dbg
        self.N = N
        self.n_cores = n_cores
        self.NL = N // 2
        self.NP = self.NL + 2 * HALO
        self.NPP = ceil_div(self.NP, 128) * 128
        self.NY = self.NL + 2 * PAD
        self.L = L
        self.GL = min(GL, self.NPP)
        self.KB = KB
        self.M = float(max(1, n_cores // 2) * N)


def build_program(ctx: ExitStack, tc: tile.TileContext, cfg: Cfg):
    nc = tc.nc
    N, NL, NP, NPP, NY, L = (
        cfg.N, cfg.NL, cfg.NP, cfg.NPP, cfg.NY, cfg.L)

    xT = nc.dram_tensor("xT", [N + 1, C], BF16, kind="ExternalInput")
    xTf = nc.dram_tensor("xTf", [N + 1, C], F32, kind="ExternalInput")
    cR = nc.dram_tensor("cR", [N + 1, 4], F32, kind="ExternalInput")
    idxh = nc.dram_tensor("idxh", [NPP, 1], I32, kind="ExternalInput")
    w1T = nc.dram_tensor("w1T", [C, K * C], BF16, kind="ExternalInput")
    w2T = nc.dram_tensor("w2T", [C, K * C], BF16, kind="ExternalInput")
    S9 = nc.dram_tensor("S9", [27, 9], BF16, kind="ExternalInput")
    Ibf = nc.dram_tensor("Ibf", [C, C], BF16, kind="ExternalInput")
    If32 = nc.dram_tensor("If32", [C, C], F32, kind="ExternalInput")
    gbT = nc.dram_tensor("gbT", [C, 4], F32, kind="ExternalInput")
    outT = nc.dram_tensor("outT", [N, C], F32, kind="ExternalOutput")

    cgTd = nc.dram_tensor("cgTd", [3, NPP], F32)
    g9d = nc.dram_tensor("g9d", [K, NPP], BF16)
    st_in = [nc.dram_tensor(f"st_in{i}", [C, 2], F32) for i in range(2)]
    st_space = "Shared" if cfg.n_cores > 4 else "Local"
    st_out = [nc.dram_tensor(f"st_out{i}", [C, 2], F32, addr_space=st_space)
              for i in range(2)]

    consts = ctx.enter_context(tc.tile_pool(name="consts", bufs=1))
    resid = ctx.enter_context(tc.tile_pool(name="resid", bufs=1))
    gpool = ctx.enter_context(tc.tile_pool(name="gath", bufs=2))
    xpool = ctx.enter_context(tc.tile_pool(name="xp", bufs=2))
    rpool = ctx.enter_context(tc.tile_pool(name="rrep", bufs=2))
    wpool = ctx.enter_context(tc.tile_pool(name="xw", bufs=2))
    spool = ctx.enter_context(tc.tile_pool(name="small", bufs=4))
    epool = ctx.enter_context(tc.tile_pool(name="evict", bufs=2))
    psum = ctx.enter_context(tc.tile_pool(name="psum", bufs=2, space="PSUM"))
    psumT = psum

    w1s = consts.tile([C, K * C], BF16)
    w2s = consts.tile([C, K * C], BF16)
    S9s = consts.tile([27, 9], BF16)
    Ibfs = consts.tile([C, C], BF16)
    If32s = consts.tile([C, C], F32)
    gbs = consts.tile([C, 4], F32)
    nc.sync.dma_start(w1s[:], w1T[:, :])
    nc.sync.dma_start(w2s[:], w2T[:, :])
    nc.sync.dma_start(S9s[:], S9[:, :])
    nc.sync.dma_start(Ibfs[:], Ibf[:, :])
    nc.sync.dma_start(If32s[:], If32[:, :])
    nc.sync.dma_start(gbs[:], gbT[:, :])

    y1s = resid.tile([C, NY], BF16)
    y2s = resid.tile([C, NL], BF16)
    NB1 = ceil_div(NY, 512)
    NB2 = ceil_div(NL, 512)
    p1sum = resid.tile([C, NB1], F32)
    p1sq = resid.tile([C, NB1], F32)
    p2sum = resid.tile([C, NB2], F32)
    p2sq = resid.tile([C, NB2], F32)
    ab1 = resid.tile([C, 2], F32)
    ab2 = resid.tile([C, 2], F32)
    onesb = resid.tile([1, C], BF16)
    nc.vector.memset(onesb[:], 1.0)

    # ---- P0: coords gather + f32 PE transpose -> spill cgT to DRAM ----
    n_cblk = NPP // 128
    for b0 in range(0, n_cblk, 4):
        nb = min(4, n_cblk - b0)
        ct_ps = psumT.tile([16, 128], F32, tag="tp")
        crows = gpool.tile([128, 16], F32, tag="crows")
        idxt = spool.tile([128, 4], I32, tag="cidx")
        for b in range(nb):
            r0 = (b0 + b) * 128
            nc.sync.dma_start(
                idxt[:, b : b + 1], idxh[r0 : r0 + 128, :])
            nc.gpsimd.indirect_dma_start(
                out=crows[:, 4 * b : 4 * b + 4],
                out_offset=None,
                in_=cR[:, :],
                in_offset=IndirectOffsetOnAxis(ap=idxt[:, b : b + 1], axis=0),
            )
        nc.tensor.matmul(
            ct_ps[: 4 * nb, :],
            lhsT=crows[:, : 4 * nb],
            rhs=If32s[:],
            start=True, stop=True,
        )
        cstage = spool.tile([16, 128], F32, tag="cstage")
        nc.vector.tensor_copy(cstage[: 4 * nb, :], ct_ps[: 4 * nb, :])
        for b in range(nb):
            r0 = (b0 + b) * 128
            nc.sync.dma_start(
                cgTd[:, r0 : r0 + 128],
                cstage[4 * b : 4 * b + 3, :])

    # ---- G: g9d[t, p] = exp(-sum_d (cgT[d, p+t-4] - cgT[d, p])^2) ----
    GL = cfg.GL
    gphase = tc.tile_pool(name="gphase", bufs=1)
    gp = gphase.__enter__()
    for a in range(0, NPP, GL):
        Lc = min(GL, NPP - a)
        cg27 = gp.tile([27, GL], F32, tag="cg27")
        b27 = gp.tile([27, GL], F32, tag="b27")
        nc.gpsimd.memset(cg27[:], 0.0)
        for t in range(K):
            s0 = a + t - PAD
            d0 = 0
            if s0 < 0:
                d0 = -s0
                s0 = 0
            ln = min(Lc - d0, NPP - s0)
            if ln <= 0:
                continue
            nc.sync.dma_start(
                cg27[3 * t : 3 * t + 3, d0 : d0 + ln],
                cgTd[:, s0 : s0 + ln])
        nc.sync.dma_start(
            b27[:, :Lc],
            cgTd.ap()
            .unsqueeze(0)
            .to_broadcast([K, 3, NPP])[:, :, a : a + Lc],
        )
        rel = gp.tile([27, GL], F32, tag="rel")
        nc.gpsimd.tensor_tensor(
            out=rel[:, :Lc], in0=cg27[:, :Lc], in1=b27[:, :Lc],
            op=ALU.subtract)
        rel2 = gp.tile([27, GL], BF16, tag="rel2")
        nc.gpsimd.tensor_tensor(
            out=rel2[:, :Lc], in0=rel[:, :Lc], in1=rel[:, :Lc],
            op=ALU.mult)
        for j in range(0, Lc, 512):
            nj = min(512, Lc - j)
            qps = psum.tile([K, 512], F32, tag="big")
            nc.tensor.matmul(
                qps[:, :nj], lhsT=S9s[:], rhs=rel2[:, j : j + nj],
                start=True, stop=True)
            gst = gp.tile([K, 512], BF16, tag="gst")
            nc.scalar.activation(gst[:, :nj], qps[:, :nj], AF.Exp)
            nc.sync.dma_start(g9d[:, a + j : a + j + nj], gst[:, :nj])
    gphase.__exit__(None, None, None)

    # ---- conv pass (conv1 / conv2) ----
    def conv_pass(src_get, wts, y_put, y_len, y_off):
        blk_i = 0
        for a in range(0, y_len, L):
            Lc = min(L, y_len - a)
            xin = src_get(a, Lc)
            ga = a + y_off - PAD
            Rts = []
            for t in range(PAD):
                Rt = rpool.tile([C, L + HALO], BF16, tag=f"R{t}")
                src = (
                    g9d.ap()[t, :]
                    .unsqueeze(0)
                    .to_broadcast([C, NPP])[:, ga : ga + Lc + HALO]
                )
                nc.sync.dma_start(Rt[:, : Lc + HALO], src)
                Rts.append(Rt)
            xws = []
            for t in range(K):
                if t == PAD:
                    xws.append(None)
                    continue
                xw = wpool.tile([C, L], BF16, tag=f"xw{t % 2}")
                tm = t if t < PAD else 8 - t
                off = PAD if t < PAD else t
                nc.vector.tensor_tensor(
                    out=xw[:, :Lc],
                    in0=xin[:, t : t + Lc],
                    in1=Rts[tm][:, off : off + Lc],
                    op=ALU.mult)
                xws.append(xw)
            for j in range(0, Lc, 512):
                nj = min(512, Lc - j)
                ops = psum.tile([C, 512], F32, tag="big")
                for t in range(K):
                    rhs = (
                        xin[:, j + PAD : j + PAD + nj]
                        if t == PAD
                        else xws[t][:, j : j + nj]
                    )
                    nc.tensor.matmul(
                        ops[:, :nj],
                        lhsT=wts[:, t * C : (t + 1) * C],
                        rhs=rhs,
                        start=(t == 0), stop=(t == K - 1))
                y_put(a + j, nj, ops[:, :nj], blk_i)
                blk_i += 1

    # ---- P1: conv1 ----
    def src1(a, Lc):
        xin = xpool.tile([C, L + HALO], BF16, tag="xp")
        nrow = Lc + HALO
        nblk = ceil_div(nrow, 128)
        idxt = spool.tile([128, L // 128 + 1], I32, tag="gidx")
        nc.sync.dma_start(
            idxt[:, :nblk],
            idxh[:, 0][a : a + 128 * nblk]
            .rearrange("(k p) -> p k", p=128))
        for b in range(nblk):
            xrows = gpool.tile([128, C], BF16, tag="xrows")
            nc.gpsimd.indirect_dma_start(
                out=xrows[:, :],
                out_offset=None,
                in_=xT[:, :],
                in_offset=IndirectOffsetOnAxis(ap=idxt[:, b : b + 1], axis=0),
            )
            rr = min(128, nrow - b * 128)
            tp = psumT.tile([C, 128], F32, tag="tp")
            nc.tensor.matmul(
                tp[:, :],
                lhsT=xrows[:, :],
                rhs=Ibfs[:],
                start=True, stop=True)
            nc.scalar.activation(
                xin[:, b * 128 : b * 128 + rr], tp[:, :rr], AF.Copy)
        return xin[:]

    def put1(j, nj, ps, blk):
        lo = max(j, PAD)
        hi = min(j + nj, PAD + NL)
        if lo > j:
            nc.scalar.activation(
                y1s[:, j : lo], ps[:, : lo - j], AF.Copy)
        if hi > lo:
            nc.scalar.activation(
                y1s[:, lo : hi], ps[:, lo - j : hi - j], AF.Copy,
                accum_out=p1sum[:, blk : blk + 1])
            sq = epool.tile([C, 512], BF16, tag="sqst")
            nc.scalar.activation(
                sq[:, : hi - lo], ps[:, lo - j : hi - j], AF.Square,
                accum_out=p1sq[:, blk : blk + 1])
        else:
            nc.vector.memset(p1sum[:, blk : blk + 1], 0.0)
            nc.vector.memset(p1sq[:, blk : blk + 1], 0.0)
        if j + nj > hi:
            nc.scalar.activation(
                y1s[:, hi : j + nj], ps[:, hi - j : nj], AF.Copy)

    conv_pass(src1, w1s, put1, NY, PAD)

    # ---- stats allreduce ----
    def allreduce_stats(psm, psq, nblk, sti, sto, ab, g_col, b_col):
        tot = spool.tile([C, 2], F32, tag="tot")
        nc.vector.tensor_reduce(
            out=tot[:, 0:1], in_=psm[:, :nblk], axis=AX.X, op=ALU.add)
        nc.vector.tensor_reduce(
            out=tot[:, 1:2], in_=psq[:, :nblk], axis=AX.X, op=ALU.add)
        nc.sync.dma_start(sti[:, :], tot[:])
        red = spool.tile([C, 2], F32, tag="red")
        if cfg.n_cores > 1:
            nc.gpsimd.collective_compute(
                "AllReduce", ALU.add,
                replica_groups=[list(range(cfg.n_cores))],
                ins=[sti.ap().opt()], outs=[sto.ap().opt()],
            )
            nc.sync.dma_start(red[:], sto[:, :])
        else:
            nc.sync.dma_start(red[:], sti[:, :])
        mv = spool.tile([C, 4], F32, tag="mv")
        inv_m = 1.0 / cfg.M
        nc.vector.tensor_scalar_mul(mv[:, 0:1], red[:, 0:1], inv_m)
        nc.vector.tensor_scalar_mul(mv[:, 1:2], red[:, 1:2], inv_m)
        nc.vector.tensor_tensor(
            out=mv[:, 2:3], in0=mv[:, 0:1], in1=mv[:, 0:1], op=ALU.mult)
        nc.vector.tensor_tensor(
            out=mv[:, 2:3], in0=mv[:, 1:2], in1=mv[:, 2:3], op=ALU.subtract)
        nc.vector.tensor_scalar_add(mv[:, 3:4], mv[:, 2:3], 1e-5)
        sqv = spool.tile([C, 2], F32, tag="sqv")
        nc.scalar.activation(sqv[:, 0:1], mv[:, 3:4], AF.Sqrt)
        nc.vector.reciprocal(sqv[:, 1:2], sqv[:, 0:1])
        nc.vector.tensor_tensor(
            out=ab[:, 0:1], in0=gbs[:, g_col : g_col + 1], in1=sqv[:, 1:2],
            op=ALU.mult)
        tmp = spool.tile([C, 1], F32, tag="tmpb")
        nc.vector.tensor_tensor(
            out=tmp[:, 0:1], in0=ab[:, 0:1], in1=mv[:, 0:1], op=ALU.mult)
        nc.vector.tensor_tensor(
            out=ab[:, 1:2], in0=gbs[:, b_col : b_col + 1], in1=tmp[:, 0:1],
            op=ALU.subtract)

    allreduce_stats(p1sum, p1sq, NB1, st_in[0], st_out[0], ab1, 0, 1)

    # ---- P2: conv2 ----
    def src2(a, Lc):
        hin = xpool.tile([C, L + HALO], BF16, tag="hp")
        nc.scalar.activation(
            hin[:, : Lc + HALO], y1s[:, a : a + Lc + HALO], AF.Relu,
            bias=ab1[:, 1:2], scale=ab1[:, 0:1])
        return hin[:]

    def put2(j, nj, ps, blk):
        nc.scalar.activation(
            y2s[:, j : j + nj], ps, AF.Copy,
            accum_out=p2sum[:, blk : blk + 1])
        sq = epool.tile([C, 512], BF16, tag="sqst")
        nc.scalar.activation(
            sq[:, :nj], ps, AF.Square,
            accum_out=p2sq[:, blk : blk + 1])

    conv_pass(src2, w2s, put2, NL, HALO)

    allreduce_stats(p2sum, p2sq, NB2, st_in[1], st_out[1], ab2, 2, 3)

    # ---- P3: bn2 + identity + relu + scatter ----
    diag2 = resid.tile([C, C], BF16)
    nc.vector.tensor_tensor(
        out=diag2[:], in0=Ibfs[:],
        in1=ab2[:, 0:1].to_broadcast([C, C]), op=ALU.mult)
    b2ps = psumT.tile([1, C], F32, tag="tp")
    nc.tensor.matmul(
        b2ps[:], lhsT=ab2[:, 1:2], rhs=If32s[:], start=True, stop=True)
    b2row = resid.tile([1, C], BF16)
    nc.vector.tensor_copy(b2row[:], b2ps[:])

    for a in range(0, NL, 512):
        Lc = min(512, NL - a)
        kb = ceil_div(Lc, 128)
        ps3 = psum.tile([C, 512], F32, tag="big")
        idxt = spool.tile([128, 4], I32, tag="sidx")
        nc.sync.dma_start(
            idxt[:, :kb],
            idxh[:, 0][HALO + a : HALO + a + 128 * kb]
            .rearrange("(k p) -> p k", p=128))
        xid = gpool.tile([128, 4 * C], F32, tag="xid")
        for b in range(kb):
            nc.gpsimd.indirect_dma_start(
                out=xid[:, b * C : (b + 1) * C],
                out_offset=None,
                in_=xTf[:, :],
                in_offset=IndirectOffsetOnAxis(ap=idxt[:, b : b + 1], axis=0),
            )
        for b in range(kb):
            nb = min(128, Lc - b * 128)
            nc.tensor.matmul(
                ps3[:, b * C : b * C + C],
                lhsT=y2s[:, a + b * 128 : a + b * 128 + nb],
                rhs=diag2[:],
                start=True, stop=False)
            nc.tensor.matmul(
                ps3[:, b * C : b * C + C],
                lhsT=onesb[:],
                rhs=b2row[:],
                start=False, stop=True)
        fin = epool.tile([128, 4 * C], F32, tag="fin")
        nc.vector.tensor_tensor(
            out=fin[:, : kb * C], in0=ps3[:, : kb * C],
            in1=xid[:, : kb * C], op=ALU.add)
        nc.vector.tensor_scalar_max(fin[:, : kb * C], fin[:, : kb * C], 0.0)
        for b in range(kb):
            nc.gpsimd.indirect_dma_start(
                out=outT[:, :],
                out_offset=IndirectOffsetOnAxis(ap=idxt[:, b : b + 1], axis=0),
                in_=fin[:, b * C : (b + 1) * C],
                in_offset=None,
            )

    if cfg.dbg:
        dcg = nc.dram_tensor("dcg", [3, NPP], F32, kind="ExternalOutput")
        dg9 = nc.dram_tensor("dg9", [K, NPP], BF16, kind="ExternalOutput")
        dy1 = nc.dram_tensor("dy1", [C, NY], BF16, kind="ExternalOutput")
        dy2 = nc.dram_tensor("dy2", [C, NL], BF16, kind="ExternalOutput")
        dab = nc.dram_tensor("dab", [C, 4], F32, kind="ExternalOutput")
        for a in range(0, NPP, 4096):
            ln = min(4096, NPP - a)
            stg = gpool.tile([27, 4096], F32, tag="dstg")
            nc.sync.dma_start(stg[:3, :ln], cgTd[:, a : a + ln])
            nc.sync.dma_start(dcg[:, a : a + ln], stg[:3, :ln])
            stg2 = gpool.tile([K, 4096], BF16, tag="dstg2")
            nc.sync.dma_start(stg2[:, :ln], g9d[:, a : a + ln])
            nc.sync.dma_start(dg9[:, a : a + ln], stg2[:, :ln])
        nc.sync.dma_start(dy1[:, :], y1s[:])
        nc.sync.dma_start(dy2[:, :], y2s[:])
        nc.sync.dma_start(dab[:, 0:2], ab1[:])
        nc.sync.dma_start(dab[:, 2:4], ab2[:])


def make_host_inputs_batch(cfg: Cfg, x, coords):
    """Per-batch tensors shared by the two cores of a batch.
    x: [C, N] f32, coords: [3, N] f32."""
    N = cfg.N
    xTf = np.concatenate(
        [np.ascontiguousarray(x.T), np.zeros((1, C), np.float32)], axis=0)
    xTb = xTf.astype(ml_dtypes.bfloat16)
    cRf = np.zeros((N + 1, 4), np.float32)
    cRf[:N, :3] = coords.T
    cRf[N, :3] = 1e3  # OOB sentinel -> g ~ exp(-1e6) = 0
    return xTb, xTf, cRf


def make_idx(cfg: Cfg, indices, core_half):
    N, NL, NPP = cfg.N, cfg.NL, cfg.NPP
    n0 = core_half * NL
    idx = np.full((NPP, 1), N, np.int32)
    lo = n0 - HALO
    for p in range(cfg.NP):
        n = lo + p
        if 0 <= n < N:
            idx[p, 0] = indices[n]
    return idx


def make_const_inputs(w1, gamma1, beta1, w2, gamma2, beta2):
    w1T = np.ascontiguousarray(
        w1.transpose(1, 2, 0).reshape(C, K * C)).astype(ml_dtypes.bfloat16)
    w2T = np.ascontiguousarray(
        w2.transpose(1, 2, 0).reshape(C, K * C)).astype(ml_dtypes.bfloat16)
    S9 = np.zeros((27, 9), np.float32)
    for t in range(K):
        if t == PAD:
            continue
        for d in range(3):
            S9[3 * t + d, t] = -1.0
    S9 = S9.astype(ml_dtypes.bfloat16)
    Ibf = np.eye(C, dtype=np.float32).astype(ml_dtypes.bfloat16)
    If32 = np.eye(C, dtype=np.float32)
    gbT = np.stack([gamma1, beta1, gamma2, beta2], axis=1).astype(np.float32)
    return {"w1T": w1T, "w2T": w2T, "S9": S9, "Ibf": Ibf, "If32": If32,
            "gbT": gbT}


_CACHE = {}
LAST_PERF = {}


def _get_nc(cfg: Cfg):
    key = (cfg.N, cfg.n_cores, cfg.L, cfg.GL, cfg.KB)
    if key in _CACHE:
        return _CACHE[key]
    nc = bacc.Bacc("TRN2", target_bir_lowering=False, debug=False,
                   num_devices=cfg.n_cores)
    with tile.TileContext(nc) as tc:
        with ExitStack() as ctx:
            build_program(ctx, tc, cfg)
    nc.compile()
    _CACHE[key] = nc
    return nc


def kernel(x, coords, indices, reindices, w1, gamma1, beta1,
           w2, gamma2, beta2, _trace=False):
    x = np.asarray(x, np.float32)
    coords = np.asarray(coords, np.float32)
    indices = np.asarray(indices, np.int32)
    w1 = np.asarray(w1, np.float32)
    w2 = np.asarray(w2, np.float32)
    B, Ch, N = x.shape
    assert Ch == C
    cfg = Cfg(N, 2 * B)
    nc = _get_nc(cfg)

    const_in = make_const_inputs(
        w1, np.asarray(gamma1, np.float32), np.asarray(beta1, np.float32),
        w2, np.asarray(gamma2, np.float32), np.asarray(beta2, np.float32))
    in_maps = []
    for b in range(B):
        xTb, xTf, cRf = make_host_inputs_batch(cfg, x[b], coords[b])
        for half in range(2):
            im = dict(const_in)
            im["xT"] = xTb
            im["xTf"] = xTf
            im["cR"] = cRf
            im["idxh"] = make_idx(cfg, indices[b], half)
            in_maps.append(im)

    res = run_bass_kernel_spmd(
        nc, in_maps, core_ids=list(range(cfg.n_cores)), trace=_trace)
    LAST_PERF.clear()
    LAST_PERF["exec_time_ns"] = res.exec_time_ns

    out = np.empty((B, C, N), np.float32)
    NL = cfg.NL
    for b in range(B):
        for half in range(2):
            o = res.results[2 * b + half]["outT"]
            rows = indices[b][half * NL : (half + 1) * NL]
            out[b][:, rows] = o[rows].T
    return out
